# revision 33
# baseline (speedup 1.0000x reference)
"""Distributed Trainium2 kernel for AttentionOneHotConv (GAT-style message passing).

Strategy (8 NeuronCores, dst-node ownership):
  - Core k owns nodes [k*2500, (k+1)*2500) and all edges whose dst lands there.
  - Phase A (local): bitonic-sort each onehot row, symlog, conv pipe as banded
    matmuls (L on partitions), readout, xin @ [lin_w | Wl | Wr] -> xh, al, ar.
  - AllGather the node table T1=[xh|al] (and T2=onehot, which starts at t0).
  - Edge phase: edges pre-sorted by dst-tile (host), padded to 128-edge tiles.
    Gather T1/T2 rows by src via batched indirect DMA; segment softmax without
    max-subtraction (shift-invariant, alphas are small); scatter-add via
    one-hot matmuls accumulating [wx | sum_exp | onehot] in PSUM per dst tile.
  - Softmax denominator is applied after aggregation (linearity).
"""

import numpy as np

N = 20000
L = 128
IN = 512
H = 8
C = 32
OHC = 8
NCORE = 8
NLOC = N // NCORE          # 2500
P = 128
NT = (NLOC + P - 1) // P   # 20 node tiles per core (last has 68 rows)
TAIL = NLOC - (NT - 1) * P  # 68
NEG_SLOPE = 0.2
PAD_AL = -1.0e9
F32 = np.float32


# ---------------------------------------------------------------------------
# bitonic sorting network pass descriptors (ascending, n=128 along free axis)
# each pass: list of (dir, base_offset) with shared dims; pairs (i, i+j)
def bitonic_passes():
    passes = []
    for s in range(7):
        for jj in range(s, -1, -1):
            passes.append((s, jj))
    return passes


def host_sort_check():
    # numpy emulation of the AP-structured network, used as a self check
    rng = np.random.default_rng(0)
    v = rng.random((4, 128)).astype(np.float32)
    a = v.copy()
    for s, jj in bitonic_passes():
        j = 1 << jj
        k = 1 << (s + 1)
        i = np.arange(128)
        sel = (i & j) == 0
        ii = i[sel]
        ll = ii + j
        asc = (ii & k) == 0
        lo = np.minimum(a[:, ii], a[:, ll])
        hi = np.maximum(a[:, ii], a[:, ll])
        na = a.copy()
        na[:, ii[asc]] = lo[:, asc]
        na[:, ll[asc]] = hi[:, asc]
        na[:, ii[~asc]] = hi[:, ~asc]
        na[:, ll[~asc]] = lo[:, ~asc]
        a = na
    assert np.allclose(a, np.sort(v, axis=-1)), "bitonic network broken"


# ---------------------------------------------------------------------------
def build_bass(T_all):
    import concourse.bass as bass
    import concourse.bacc as bacc
    import concourse.tile as tile
    from concourse import mybir
    from concourse.masks import make_identity

    f32 = mybir.dt.float32
    i32 = mybir.dt.int32
    AF = mybir.ActivationFunctionType
    OP = mybir.AluOpType

    nc = bacc.Bacc("TRN2", target_bir_lowering=False, debug=False,
                   num_devices=NCORE, num_swdge_queues=1)

    # ---- I/O parameters (per core shards / replicated small weights)
    x_p = nc.declare_dram_parameter("x", [NLOC, IN], f32, isOutput=False)
    oh_p = nc.declare_dram_parameter("oh", [NLOC, L], f32, isOutput=False)
    linext_p = nc.declare_dram_parameter("linext", [IN + OHC, 272], f32, isOutput=False)
    w1rep_p = nc.declare_dram_parameter("w1rep", [P, 3, 8], f32, isOutput=False)
    w2rep_p = nc.declare_dram_parameter("w2rep", [P, 3, 128], f32, isOutput=False)
    scal_p = nc.declare_dram_parameter("scalrep", [P, 32], f32, isOutput=False)
    lin2w_p = nc.declare_dram_parameter("lin2w", [16, 8], f32, isOutput=False)
    lin2b_p = nc.declare_dram_parameter("lin2b", [8, 1], f32, isOutput=False)
    biasrep_p = nc.declare_dram_parameter("biasrep", [P, 256], f32, isOutput=False)
    gidx_p = nc.declare_dram_parameter("gidx", [NT, P, T_all], i32, isOutput=False)
    drow_p = nc.declare_dram_parameter("drows", [NT, 1, T_all * P], f32, isOutput=False)
    dcol_p = nc.declare_dram_parameter("dcols", [NT, P, T_all], f32, isOutput=False)
    out_p = nc.declare_dram_parameter("out", [NLOC, 384], f32, isOutput=True)

    NROW1 = 264   # [xh(256) | al(8)]
    T1W = 264     # T1 row width
    TROWS = N + 8

    with tile.TileContext(nc) as tc:
        import contextlib
        ctx = contextlib.ExitStack()
        with ctx:
            dram = ctx.enter_context(tc.tile_pool(name="dram", bufs=1, space="DRAM"))
            const = ctx.enter_context(tc.tile_pool(name="const", bufs=1))

            # ---- internal DRAM for collectives / gather tables
            xh_local = dram.tile([NLOC, T1W], f32)
            oh_bounce = dram.tile([NLOC, L], f32)
            T1 = dram.tile([TROWS, T1W], f32)
            T2 = dram.tile([TROWS, L], f32)

            # ---- kick off onehot allgather immediately
            nc.sync.dma_start(oh_bounce[:], oh_p.ap())
            nc.gpsimd.collective_compute(
                "AllGather", OP.bypass,
                replica_groups=[list(range(NCORE))],
                ins=[oh_bounce[:].opt()],
                outs=[T2[0:N, :].opt()],
            )
            # T2 pad rows = 0
            zrow = const.tile([8, L], f32)
            nc.vector.memset(zrow[:], 0.0)
            nc.sync.dma_start(T2[N:TROWS, :], zrow[:])
            # T1 pad rows: xh=0, al=-1e9
            prow = const.tile([8, T1W], f32)
            nc.vector.memset(prow[:], 0.0)
            nc.vector.memset(prow[:, 256:264], PAD_AL)
            nc.sync.dma_start(T1[N:TROWS, :], prow[:])

            # ---- constants
            identity = const.tile([P, P], f32)
            make_identity(nc, identity[:])

            iota_row_i = const.tile([P, P], i32)
            nc.gpsimd.iota(iota_row_i[:], pattern=[[1, P]], base=0, channel_multiplier=0)
            iota_row = const.tile([P, P], f32)
            nc.vector.tensor_copy(iota_row[:], iota_row_i[:])

            iota_col_i = const.tile([P, 1], i32)
            nc.gpsimd.iota(iota_col_i[:], pattern=[[0, 1]], base=0, channel_multiplier=1)
            iota_col = const.tile([P, 1], f32)
            nc.vector.tensor_copy(iota_col[:], iota_col_i[:])

            # k-index for conv band masks: val = l' - l + 1
            kidx_i = const.tile([P, P], i32)
            nc.gpsimd.iota(kidx_i[:], pattern=[[1, P]], base=1, channel_multiplier=-1)
            kidx = const.tile([P, P], f32)
            nc.vector.tensor_copy(kidx[:], kidx_i[:])

            ones1 = const.tile([1, P], f32)
            nc.vector.memset(ones1[:], 1.0)
            ones_col = const.tile([P, 1], f32)
            nc.vector.memset(ones_col[:], 1.0)

            # ind16[:, o2*16+m] = (m == o2), used to place ones-reductions in rows
            ind16_i = const.tile([P, 256], i32)
            nc.gpsimd.iota(ind16_i[:], pattern=[[1, 16], [-1, 16]], base=0,
                           channel_multiplier=0)
            ind16 = const.tile([P, 256], f32)
            nc.vector.tensor_scalar(out=ind16[:], in0=ind16_i[:], scalar1=0,
                                    scalar2=None, op0=OP.is_equal)

            # ---- small weights to SBUF
            w1rep = const.tile([P, 3, 8], f32)
            nc.sync.dma_start(w1rep[:], w1rep_p.ap())
            w2rep = const.tile([P, 3, 128], f32)
            nc.sync.dma_start(w2rep[:], w2rep_p.ap())
            scal = const.tile([P, 32], f32)
            nc.sync.dma_start(scal[:], scal_p.ap())
            lin2w = const.tile([16, 8], f32)
            nc.sync.dma_start(lin2w[:], lin2w_p.ap())
            lin2b = const.tile([8, 1], f32)
            nc.sync.dma_start(lin2b[:], lin2b_p.ap())
            biasrep = const.tile([P, 256], f32)
            nc.sync.dma_start(biasrep[:], biasrep_p.ap())
            linext = const.tile([P, 4, 272], f32)
            nc.sync.dma_start(linext[:], linext_p.ap()[0:512, :].rearrange("(c p) m -> p c m", p=P))
            linro = const.tile([8, 272], f32)
            nc.sync.dma_start(linro[:], linext_p.ap()[512:520, :])

            # ---- onehot raw, resident for edge phase epilogue
            oh_raw = const.tile([P, NT, L], f32)
            nc.vector.memset(oh_raw[:, NT - 1, :], 0.0)
            nc.sync.dma_start(
                oh_raw[:, 0:NT - 1, :],
                oh_p.ap()[0:(NT - 1) * P, :].rearrange("(t p) l -> p t l", p=P))
            nc.sync.dma_start(
                oh_raw[0:TAIL, NT - 1, :],
                oh_p.ap()[(NT - 1) * P:NLOC, :].rearrange("(t p) l -> p t l", p=TAIL))

            # ---- conv band masks (3x [P,P]) and band stacks
            with tc.tile_pool(name="bands", bufs=1) as bandpool:
                masks = []
                for k in range(3):
                    mk = bandpool.tile([P, P], f32, name=f"mask{k}")
                    nc.vector.tensor_scalar(out=mk[:], in0=kidx[:], scalar1=float(k),
                                            scalar2=None, op0=OP.is_equal)
                    masks.append(mk)

                B1 = bandpool.tile([P, 8, P], f32)
                tmp1 = bandpool.tile([P, 8, P], f32)
                for k in range(3):
                    dstt = B1 if k == 0 else tmp1
                    nc.vector.tensor_tensor(
                        out=dstt[:],
                        in0=w1rep[:, k, :, None].broadcast_to([P, 8, P]),
                        in1=masks[k][:, None, :].broadcast_to([P, 8, P]),
                        op=OP.mult)
                    if k > 0:
                        nc.vector.tensor_tensor(out=B1[:], in0=B1[:], in1=tmp1[:], op=OP.add)

                B2 = bandpool.tile([P, 128, P], f32)
                NCH = 4  # build in 4 chunks of 32 (o2,cin) pairs
                for cch in range(NCH):
                    sl = slice(cch * 32, (cch + 1) * 32)
                    tmp2 = bandpool.tile([P, 32, P], f32, name="b2tmp", tag="b2tmp")
                    for k in range(3):
                        dstt = B2[:, sl, :] if k == 0 else tmp2[:]
                        nc.vector.tensor_tensor(
                            out=dstt,
                            in0=w2rep[:, k, sl, None].broadcast_to([P, 32, P]),
                            in1=masks[k][:, None, :].broadcast_to([P, 32, P]),
                            op=OP.mult)
                        if k > 0:
                            nc.vector.tensor_tensor(out=B2[:, sl, :], in0=B2[:, sl, :],
                                                    in1=tmp2[:], op=OP.add)

                # ---- sort: all 20 tiles at once, ping-pong buffers
                with tc.tile_pool(name="sortp", bufs=1) as sortp:
                    sbufs = [sortp.tile([P, NT, L], f32, name=f"sort{i}") for i in range(2)]
                    cur = oh_raw
                    nxt = sbufs[0]
                    for (s, jj) in bitonic_passes():
                        j = 1 << jj
                        m_cnt = 1 << (s - jj)
                        if s < 6:
                            va = cur[:].rearrange("p t (u dir m two r) -> p (t u) dir m two r",
                                                  dir=2, m=m_cnt, two=2, r=j)
                            vb = nxt[:].rearrange("p t (u dir m two r) -> p (t u) dir m two r",
                                                  dir=2, m=m_cnt, two=2, r=j)
                            for dirv in (0, 1):
                                a0 = va[:, :, dirv, :, 0, :]
                                a1 = va[:, :, dirv, :, 1, :]
                                b0 = vb[:, :, dirv, :, 0, :]
                                b1 = vb[:, :, dirv, :, 1, :]
                                lo_op = OP.min if dirv == 0 else OP.max
                                hi_op = OP.max if dirv == 0 else OP.min
                                nc.vector.tensor_tensor(out=b0, in0=a0, in1=a1, op=lo_op)
                                nc.vector.tensor_tensor(out=b1, in0=a0, in1=a1, op=hi_op)
                        else:
                            va = cur[:].rearrange("p t (m two r) -> p (t m) two r",
                                                  m=m_cnt, two=2, r=j)
                            vb = nxt[:].rearrange("p t (m two r) -> p (t m) two r",
                                                  m=m_cnt, two=2, r=j)
                            nc.vector.tensor_tensor(out=vb[:, :, 0, :], in0=va[:, :, 0, :],
                                                    in1=va[:, :, 1, :], op=OP.min)
                            nc.vector.tensor_tensor(out=vb[:, :, 1, :], in0=va[:, :, 0, :],
                                                    in1=va[:, :, 1, :], op=OP.max)
                        if cur is oh_raw:
                            cur, nxt = sbufs[0], sbufs[1]
                        else:
                            cur, nxt = nxt, cur
                    sorted_all = cur
                    # symlog: onehot values are in [0,1): symlog = ln(1+v)
                    nc.scalar.activation(sorted_all[:], sorted_all[:], AF.Ln, bias=1.0)

                    # ---- Phase A per node tile
                    ar_all = const.tile([P, NT * 8], f32)
                    with tc.tile_pool(name="phA", bufs=2) as ph, \
                         tc.tile_pool(name="phAp", bufs=1, space="PSUM") as php:
                        for t in range(NT):
                            rows = P if t < NT - 1 else TAIL
                            # transpose sorted tile -> [L, n]
                            pT_ps = php.tile([P, P], f32, name="pT_ps", tag="tr_ps")
                            nc.tensor.transpose(pT_ps[:], sorted_all[:, t, :], identity[:])
                            pT = ph.tile([P, P], f32, name="pT", tag="pT")
                            nc.vector.tensor_copy(pT[:], pT_ps[:])

                            # conv1: 8 output channels
                            c1_ps = php.tile([P, 8, P], f32, name="c1_ps", tag="c1_ps")
                            for o in range(8):
                                nc.tensor.matmul(c1_ps[:, o, :], B1[:, o, :], pT[:],
                                                 start=True, stop=True)
                            h1 = ph.tile([P, 8, P], f32, name="h1", tag="h1")
                            for o in range(8):
                                nc.scalar.activation(h1[:, o, :], c1_ps[:, o, :], AF.Relu,
                                                     bias=scal[:, o:o + 1])

                            # conv2: 16 out channels, accumulate over cin
                            h2 = ph.tile([P, 16, P], f32, name="h2", tag="h2")
                            for o2 in range(16):
                                c2_ps = php.tile([P, P], f32, name="c2_ps", tag="c2_ps")
                                for cin in range(8):
                                    nc.tensor.matmul(c2_ps[:], B2[:, o2 * 8 + cin, :],
                                                     h1[:, cin, :],
                                                     start=(cin == 0), stop=(cin == 7))
                                nc.scalar.activation(h2[:, o2, :], c2_ps[:], AF.Relu,
                                                     bias=scal[:, 8 + o2:9 + o2])

                            # mean over L (indicator-column matmuls accumulate rows;
                            # 1/128 folded into lin2w)
                            mean_ps = php.tile([16, P], f32, name="mean_ps", tag="mean_ps")
                            for o2 in range(16):
                                nc.tensor.matmul(mean_ps[:], ind16[:, o2 * 16:(o2 + 1) * 16],
                                                 h2[:, o2, :],
                                                 start=(o2 == 0), stop=(o2 == 15))
                            mean_sb = ph.tile([16, P], f32, name="mean_sb", tag="mean_sb")
                            nc.vector.tensor_copy(mean_sb[:], mean_ps[:])

                            ro_ps = php.tile([8, P], f32, name="ro_ps", tag="ro_ps")
                            nc.tensor.matmul(ro_ps[:], lin2w[:], mean_sb[:],
                                             start=True, stop=True)
                            roT = ph.tile([8, P], f32, name="roT", tag="roT")
                            nc.scalar.activation(roT[:], ro_ps[:], AF.Identity, bias=lin2b[:])

                            # x transpose (4 chunks of 128 features)
                            x_t = ph.tile([P, IN], f32, name="x_t", tag="x_t")
                            if rows < P:
                                nc.vector.memset(x_t[:], 0.0)
                            nc.sync.dma_start(x_t[0:rows, :], x_p.ap()[t * P:t * P + rows, :])
                            xT = ph.tile([P, 4, P], f32, name="xT", tag="xT")
                            for cch in range(4):
                                xT_ps = php.tile([P, P], f32, name="xT_ps", tag="tr_ps")
                                nc.tensor.transpose(xT_ps[:], x_t[:, cch * P:(cch + 1) * P],
                                                    identity[:])
                                nc.vector.tensor_copy(xT[:, cch, :], xT_ps[:])

                            # xh_ext = xin @ [lin_w | Wl | Wr]
                            xh_ps = php.tile([P, 272], f32, name="xh_ps", tag="xh_ps")
                            for cch in range(4):
                                nc.tensor.matmul(xh_ps[:], xT[:, cch, :], linext[:, cch, :],
                                                 start=(cch == 0), stop=False)
                            nc.tensor.matmul(xh_ps[:], roT[:], linro[:],
                                             start=False, stop=True)

                            t1row = ph.tile([P, T1W], f32, name="t1row", tag="t1row")
                            nc.vector.tensor_copy(t1row[:, 0:NROW1], xh_ps[:, 0:NROW1])
                            nc.sync.dma_start(xh_local[t * P:t * P + rows, :], t1row[0:rows, :])
                            nc.vector.tensor_copy(ar_all[:, t * 8:(t + 1) * 8],
                                                  xh_ps[:, 264:272])

            # ---- allgather xh/al table
            nc.gpsimd.collective_compute(
                "AllGather", OP.bypass,
                replica_groups=[list(range(NCORE))],
                ins=[xh_local[:].opt()],
                outs=[T1[0:N, :].opt()],
            )

            # ---- edge phase
            CH = 512  # psum chunk for Mt build
            nmtch = (T_all * P + CH - 1) // CH
            with tc.tile_pool(name="edge", bufs=2) as ep, \
                 tc.tile_pool(name="edgp", bufs=2, space="PSUM") as epp, \
                 tc.tile_pool(name="edgj", bufs=3) as ej, \
                 tc.tile_pool(name="edgjp", bufs=2, space="PSUM") as ejp:
                for t in range(NT):
                    rows = P if t < NT - 1 else TAIL
                    # HW-verified gather path: the SWDGE ucode only handles one
                    # index per partition per indirect DMA (large dma_gather
                    # batches crash the runtime), so gather edge tiles one at
                    # a time from the allgathered tables.
                    idxt = ep.tile([P, T_all], i32, name="idxt", tag="idxt")
                    nc.sync.dma_start(idxt[:], gidx_p.ap()[t, :, :])
                    g1 = ep.tile([P, T_all, T1W], f32, name="g1", tag="g1")
                    g2 = ep.tile([P, T_all, L], f32, name="g2", tag="g2")
                    for j in range(T_all):
                        nc.gpsimd.indirect_dma_start(
                            out=g1[:, j, 0:NROW1], out_offset=None,
                            in_=T1[:],
                            in_offset=bass.IndirectOffsetOnAxis(
                                ap=idxt[:, j:j + 1], axis=0))
                        nc.gpsimd.indirect_dma_start(
                            out=g2[:, j, :], out_offset=None,
                            in_=T2[:],
                            in_offset=bass.IndirectOffsetOnAxis(
                                ap=idxt[:, j:j + 1], axis=0))

                    drow = ep.tile([1, T_all * P], f32, name="drow", tag="drow")
                    nc.sync.dma_start(drow[:], drow_p.ap()[t, :, :])
                    dcol = ep.tile([P, T_all], f32, name="dcol", tag="dcol")
                    nc.sync.dma_start(dcol[:], dcol_p.ap()[t, :, :])

                    # Mt for all edge tiles of this dst tile
                    mt_all = ep.tile([P, T_all * P], f32, name="mt_all", tag="mt_all")
                    for cc in range(nmtch):
                        c0 = cc * CH
                        c1 = min((cc + 1) * CH, T_all * P)
                        rep_ps = epp.tile([P, CH], f32, name="rep_ps", tag="rep_ps")
                        nc.tensor.matmul(rep_ps[:, 0:c1 - c0], ones1[:], drow[:, c0:c1],
                                         start=True, stop=True)
                        nc.vector.tensor_scalar(out=mt_all[:, c0:c1],
                                                in0=rep_ps[:, 0:c1 - c0],
                                                scalar1=iota_col[:], scalar2=None,
                                                op0=OP.is_equal)

                    agg1_ps = epp.tile([P, 264], f32, name="agg1_ps", tag="agg1_ps")
                    agg2_ps = epp.tile([P, L], f32, name="agg2_ps", tag="agg2_ps")
                    for j in range(T_all):
                        mj = ej.tile([P, P], f32, name="mj", tag="mj")
                        nc.vector.tensor_scalar(out=mj[:], in0=iota_row[:],
                                                scalar1=dcol[:, j:j + 1], scalar2=None,
                                                op0=OP.is_equal)
                        are_ps = ejp.tile([P, 8], f32, name="are_ps", tag="are_ps")
                        nc.tensor.matmul(are_ps[:], mt_all[:, j * P:(j + 1) * P],
                                         ar_all[:, t * 8:(t + 1) * 8],
                                         start=True, stop=True)
                        # alpha = leaky_relu(al + ar); ex = exp(alpha) -> into g1 al slot
                        asum = ej.tile([P, 8], f32, name="asum", tag="asum")
                        nc.vector.tensor_tensor(out=asum[:], in0=g1[:, j, 256:264],
                                                in1=are_ps[:], op=OP.add)
                        apos = ej.tile([P, 8], f32, name="apos", tag="apos")
                        nc.vector.tensor_scalar(out=apos[:], in0=asum[:], scalar1=0.0,
                                                scalar2=None, op0=OP.max)
                        nc.vector.tensor_scalar(out=asum[:], in0=asum[:], scalar1=0.0,
                                                scalar2=NEG_SLOPE, op0=OP.min, op1=OP.mult)
                        nc.vector.tensor_tensor(out=asum[:], in0=asum[:], in1=apos[:],
                                                op=OP.add)
                        nc.scalar.activation(g1[:, j, 256:264], asum[:], AF.Exp)
                        # wx = xh * ex (broadcast per head)
                        nc.vector.tensor_tensor(
                            out=g1[:, j, 0:256].rearrange("p (h c) -> p h c", h=8),
                            in0=g1[:, j, 0:256].rearrange("p (h c) -> p h c", h=8),
                            in1=g1[:, j, 256:264][:, :, None].broadcast_to([P, 8, 32]),
                            op=OP.mult)
                        nc.tensor.matmul(agg1_ps[:], mj[:], g1[:, j, 0:264],
                                         start=(j == 0), stop=(j == T_all - 1))
                        nc.tensor.matmul(agg2_ps[:], mj[:], g2[:, j, :],
                                         start=(j == 0), stop=(j == T_all - 1))

                    # epilogue: divide by denom, add bias / onehot residual
                    recip = ep.tile([P, 8], f32, name="recip", tag="recip")
                    nc.vector.reciprocal(recip[:], agg1_ps[:, 256:264])
                    osb = ep.tile([P, 384], f32, name="osb", tag="osb")
                    nc.vector.tensor_tensor(
                        out=osb[:, 0:256].rearrange("p (h c) -> p h c", h=8),
                        in0=agg1_ps[:, 0:256].rearrange("p (h c) -> p h c", h=8),
                        in1=recip[:, :, None].broadcast_to([P, 8, 32]),
                        op=OP.mult)
                    nc.vector.tensor_tensor(out=osb[:, 0:256], in0=osb[:, 0:256],
                                            in1=biasrep[:], op=OP.add)
                    nc.vector.tensor_tensor(out=osb[:, 256:384], in0=agg2_ps[:],
                                            in1=oh_raw[:, t, :], op=OP.add)
                    nc.sync.dma_start(out_p.ap()[t * P:t * P + rows, :], osb[0:rows, :])

    nc.finalize()
    return nc


# ---------------------------------------------------------------------------
def prep_host(x, onehot, adj, lin_w, att_l, att_r, bias,
              conv1_w, conv1_b, conv2_w, conv2_b, lin2_w, lin2_b):
    x = np.asarray(x, F32)
    onehot = np.asarray(onehot, F32)
    adj = np.asarray(adj, np.int64)
    src = np.concatenate([adj[0], np.arange(N, dtype=np.int64)]).astype(np.int32)
    dst = np.concatenate([adj[1], np.arange(N, dtype=np.int64)]).astype(np.int32)

    # per (core, dst-tile) edge lists
    counts = np.zeros((NCORE, NT), np.int64)
    per = [[None] * NT for _ in range(NCORE)]
    owner = dst // NLOC
    dloc = dst - owner * NLOC
    for k in range(NCORE):
        selk = owner == k
        sk, dk = src[selk], dloc[selk]
        t_of = dk // P
        order = np.argsort(t_of * 4096 + 0, kind="stable")  # keep stable by tile
        # simpler: bucket per tile
        for t in range(NT):
            m = t_of == t
            per[k][t] = (sk[m], dk[m] - t * P)
            counts[k, t] = m.sum()
    T_all = int((counts.max() + P - 1) // P)

    # fold attention weights
    lwr = np.asarray(lin_w, F32).reshape(IN + OHC, H, C)
    Wl = np.einsum("fhc,hc->fh", lwr, np.asarray(att_l, F32)[0])
    Wr = np.einsum("fhc,hc->fh", lwr, np.asarray(att_r, F32)[0])
    linext = np.concatenate([np.asarray(lin_w, F32), Wl, Wr], axis=1).astype(F32)

    w1rep = np.broadcast_to(
        np.asarray(conv1_w, F32)[:, 0, :].T[None], (P, 3, 8)).copy()  # [p,k,o]
    w2rep = np.broadcast_to(
        np.asarray(conv2_w, F32).reshape(128, 3).T[None], (P, 3, 128)).copy()  # [p,k,(o2,cin)]
    scal = np.zeros((P, 32), F32)
    scal[:, 0:8] = np.asarray(conv1_b, F32)[None]
    scal[:, 8:24] = np.asarray(conv2_b, F32)[None]
    lin2w = (np.asarray(lin2_w, F32) / np.float32(L)).astype(F32)
    lin2b = np.asarray(lin2_b, F32).reshape(8, 1)
    biasrep = np.broadcast_to(np.asarray(bias, F32)[None], (P, 256)).copy()

    in_maps = []
    for k in range(NCORE):
        gidx = np.full((NT, P, T_all), N, np.int32)
        drows = np.zeros((NT, 1, T_all * P), F32)
        dcols = np.zeros((NT, P, T_all), F32)
        for t in range(NT):
            sk, dl = per[k][t]
            m = len(sk)
            i = np.arange(m)
            jj = i // P
            pp = i % P
            gidx[t, pp, jj] = sk
            drows[t, 0, jj * P + pp] = dl.astype(F32)
            dcols[t, pp, jj] = dl.astype(F32)
        in_maps.append({
            "x": np.ascontiguousarray(x[k * NLOC:(k + 1) * NLOC]),
            "oh": np.ascontiguousarray(onehot[k * NLOC:(k + 1) * NLOC]),
            "linext": linext,
            "w1rep": w1rep, "w2rep": w2rep, "scalrep": scal,
            "lin2w": lin2w, "lin2b": lin2b, "biasrep": biasrep,
            "gidx": gidx, "drows": drows, "dcols": dcols,
        })
    return in_maps, T_all


_CACHE = {}


def _ensure_ntff_hook():
    """The agent image's antenv lacks axon_hooks; synthesize it so
    run_bass_kernel_spmd(trace=True) can capture NTFF profiles."""
    import sys as _sys
    import types as _types
    try:
        from antenv.axon_hooks import get_axon_ntff_profile_hook  # noqa: F401
        return
    except ImportError:
        pass
    try:
        import antenv
        from trn_agent_boot.trn_boot import _ntff_profile_via_ctypes
        hook = _ntff_profile_via_ctypes("/opt/axon/libaxon_pjrt.so")
        mod = _types.ModuleType("antenv.axon_hooks")
        mod.get_axon_ntff_profile_hook = lambda: hook
        mod.set_axon_ntff_profile_hook = lambda h: None
        _sys.modules["antenv.axon_hooks"] = mod
        antenv.axon_hooks = mod
    except Exception as e:  # profiling is best-effort
        print(f"ntff hook setup failed: {e}")


def kernel(x, onehot, adj, n_nodes, lin_w, att_l, att_r, bias,
           conv1_w, conv1_b, conv2_w, conv2_b, lin2_w, lin2_b,
           trace=False):
    from concourse import bass_utils
    if trace:
        _ensure_ntff_hook()

    in_maps, T_all = prep_host(x, onehot, adj, lin_w, att_l, att_r, bias,
                               conv1_w, conv1_b, conv2_w, conv2_b, lin2_w, lin2_b)
    if T_all not in _CACHE:
        _CACHE[T_all] = build_bass(T_all)
    nc = _CACHE[T_all]
    res = bass_utils.run_bass_kernel_spmd(
        nc, in_maps, core_ids=list(range(NCORE)), trace=trace)
    outs = [res.results[k]["out"] for k in range(NCORE)]
    full = np.concatenate(outs, axis=0)
    kernel.last_exec_ns = res.exec_time_ns
    kernel.last_results = res
    return full[:, 0:256].copy(), full[:, 256:384].copy()
